# revision 10
# baseline (speedup 1.0000x reference)
"""BiMamba TRN2 kernel: 8 cores = 4 samples x {forward, backward} stacks.

Each core runs the full 5-layer Mamba+MLP stack for one (sample, direction)
pair in a single Bass/Tile program (channel-major layout, fp16 compute,
fp32 scan state) and returns per-channel sums over T.  Host does embedding
gather, weight packing, and the final (sum_f + sum_b)/T combine - both
directions' means are reversal-invariant so no reordering is needed.
"""

from contextlib import ExitStack

import numpy as np

import concourse.bass as bass
import concourse.mybir as mybir
import concourse.tile as tile
from concourse import bacc
from concourse.bass_utils import run_bass_kernel_spmd

F16 = mybir.dt.float16
F32 = mybir.dt.float32
AF = mybir.ActivationFunctionType
OP = mybir.AluOpType

T = 2048
HALF = 1024
NT = 512  # psum tile width
DF = 352          # d_full; 3 channel blocks (128,128,96)
DI = 704          # d_inner; 6 channel blocks (5x128, 64)
DTR = 22
S = 16
DEPTH = 5
NB_F = 3          # d_full blocks
NB_I = 6          # d_inner blocks
FB = [128, 128, 96]    # valid channels per d_full block


def _emit(ctx, tc, dr):
    nc = tc.nc

    pool = ctx.enter_context(tc.tile_pool(name="main", bufs=1))
    p2 = ctx.enter_context(tc.tile_pool(name="dbl", bufs=2))
    trans = ctx.enter_context(tc.tile_pool(name="trans", bufs=2))
    psum = ctx.enter_context(tc.tile_pool(name="ps", bufs=4, space="PSUM"))
    psAB = ctx.enter_context(tc.tile_pool(name="psAB", bufs=2, space="PSUM"))
    dram = ctx.enter_context(tc.tile_pool(name="dram", bufs=2, space="DRAM"))

    # ---- persistent tiles ----
    resid = pool.tile([128, NB_F, T], F16, tag="resid", name="resid")
    ones = pool.tile([128, 128], F16, tag="ones", name="ones")
    nc.sync.dma_start(out=resid, in_=dr["inp"][:, :, :])
    nc.sync.dma_start(out=ones, in_=dr["ones128"][:, :])

    hcarry = pool.tile([128, NB_I * S], F32, tag="hcarry", name="hcarry")
    eps = pool.tile([128, 1], F32, tag="eps", name="eps")
    nc.vector.memset(eps, 1e-5)

    def layernorm(w, gk, bk, half, xn):
        """LN over channels of resid[half] -> xn fp16 [128, NB_F, HALF]."""
        h0 = half * HALF
        for nt in range(HALF // NT):
            sl = slice(h0 + nt * NT, h0 + (nt + 1) * NT)
            lsl = slice(nt * NT, (nt + 1) * NT)
            pA = psAB.tile([128, NT], F32, tag="pA", name="pA")
            pB = psAB.tile([128, NT], F32, tag="pB", name="pB")
            sq = trans.tile([128, NB_F, NT], F16, tag="sq", name="sq")
            for b in range(NB_F):
                k = FB[b]
                nc.scalar.activation(sq[:k, b, :], resid[:k, b, sl], AF.Square)
            for b in range(NB_F):
                k = FB[b]
                nc.tensor.matmul(pA, ones[:k, :], resid[:k, b, sl],
                                 start=(b == 0), stop=(b == NB_F - 1))
            for b in range(NB_F):
                k = FB[b]
                nc.tensor.matmul(pB, ones[:k, :], sq[:k, b, :],
                                 start=(b == 0), stop=(b == NB_F - 1))
            m16 = trans.tile([128, NT], F16, tag="m16", name="m16")
            nc.scalar.activation(m16, pA, AF.Copy, scale=1.0 / DF)
            msq = trans.tile([128, NT], F32, tag="msq", name="msq")
            nc.scalar.activation(msq, pA, AF.Square, scale=1.0 / DF)
            nc.vector.scalar_tensor_tensor(msq, pB, 1.0 / DF, msq,
                                           OP.mult, OP.subtract)
            nc.scalar.activation(msq, msq, AF.Ln, bias=eps[:, 0:1], scale=1.0)
            r16 = trans.tile([128, NT], F16, tag="r16", name="r16")
            nc.scalar.activation(r16, msq, AF.Exp, scale=-0.5)
            for b in range(NB_F):
                d1 = trans.tile([128, NT], F16, tag="lnd1", name="lnd1")
                nc.vector.tensor_sub(d1, resid[:, b, sl], m16)
                nc.vector.tensor_mul(d1, d1, r16)
                nc.vector.tensor_scalar(xn[:, b, lsl], d1,
                                        w[gk][:, b:b + 1], w[bk][:, b:b + 1],
                                        OP.mult, OP.add)

    def load_layer_weights(l):
        w = {}
        for name, shape, dt in [
            ("Win", [128, NB_F, 1536], F16), ("bin", [128, 12], F32),
            ("Wconv", [128, NB_I, 4], F32), ("bconv", [128, NB_I], F32),
            ("Dvec", [128, NB_I], F32),
            ("Wx", [128, NB_I, 54], F16),
            ("Wdt", [22, 768], F16), ("bdt", [128, NB_I], F32),
            ("Wout", [128, NB_I, 384], F16), ("bout", [128, NB_F], F32),
            ("Wm1", [128, NB_F, 768], F16), ("bm1", [128, NB_I], F32),
            ("bm1s", [128, NB_I], F32),
            ("Wm2", [128, NB_I, 384], F16), ("bm2", [128, NB_F], F32),
            ("g1", [128, NB_F], F32), ("b1", [128, NB_F], F32),
            ("g2", [128, NB_F], F32), ("b2", [128, NB_F], F32),
        ]:
            t = pool.tile(shape, dt, tag=f"w_{name}")
            nc.sync.dma_start(out=t, in_=dr[name][l])
            w[name] = t
        return w

    for l in range(DEPTH):
        w = load_layer_weights(l)

        prev_xzpad = None
        for half in range(2):
            h0 = half * HALF
            # ---- LN1 ----
            xn = pool.tile([128, NB_F, HALF], F16, tag="xn", name="xn")
            layernorm(w, "g1", "b1", half, xn)

            # ---- in_proj: xn @ Win -> xzpad (x half) and sz (silu'd z) ----
            xzpad = [p2.tile([128, HALF + 3], F16, tag=f"xzp{b}", name=f"xzp{b}")
                     for b in range(NB_I)]
            sz = [pool.tile([128, HALF], F16, tag=f"sz{b}", name=f"sz{b}") for b in range(NB_I)]
            for nt in range(HALF // NT):
                lsl = slice(nt * NT, (nt + 1) * NT)
                for m in range(12):
                    pm = psum.tile([128, NT], F32, tag="mm", name="mm")
                    for b in range(NB_F):
                        k = FB[b]
                        nc.tensor.matmul(pm, w["Win"][:k, b, m * 128:(m + 1) * 128],
                                         xn[:k, b, lsl],
                                         start=(b == 0), stop=(b == NB_F - 1))
                    if m < 6:
                        nc.scalar.activation(
                            xzpad[m][:, 3 + nt * NT: 3 + (nt + 1) * NT], pm,
                            AF.Identity, bias=w["bin"][:, m:m + 1])
                    else:
                        zp = trans.tile([128, NT], F16, tag="zp", name="zp")
                        nc.scalar.activation(zp, pm, AF.Identity,
                                             bias=w["bin"][:, m:m + 1])
                        sg = trans.tile([128, NT], F16, tag="sg", name="sg")
                        nc.scalar.activation(sg, pm, AF.Sigmoid,
                                             bias=w["bin"][:, m:m + 1])
                        nc.vector.tensor_mul(sz[m - 6][:, lsl], zp, sg)

            # conv history columns
            for b in range(NB_I):
                if half == 0:
                    nc.vector.memset(xzpad[b][:, 0:3], 0.0)
                else:
                    nc.vector.tensor_copy(xzpad[b][:, 0:3],
                                          prev_xzpad[b][:, HALF:HALF + 3])
            prev_xzpad = xzpad

            # ---- depthwise causal conv + silu -> xh ----
            xh = [pool.tile([128, HALF], F16, tag=f"xh{b}", name=f"xh{b}") for b in range(NB_I)]
            for b in range(NB_I):
                a0 = trans.tile([128, HALF], F16, tag="cva", name="cva")
                nc.vector.tensor_scalar(a0, xzpad[b][:, 0:HALF],
                                        w["Wconv"][:, b, 0:1],
                                        w["bconv"][:, b:b + 1], OP.mult, OP.add)
                prev = a0
                for k in range(1, 4):
                    ak = trans.tile([128, HALF], F16, tag=f"cv{k % 2}", name=f"cv{k % 2}")
                    nc.vector.scalar_tensor_tensor(
                        ak, xzpad[b][:, k:k + HALF], w["Wconv"][:, b, k:k + 1],
                        prev, OP.mult, OP.add)
                    prev = ak
                sg = trans.tile([128, HALF], F16, tag="cva", name="cvsg")
                nc.scalar.activation(sg, prev, AF.Sigmoid)
                nc.vector.tensor_mul(xh[b], prev, sg)

            # ---- x_proj -> xdbl [54, HALF] ----
            xdbl = pool.tile([128, HALF], F16, tag="xdbl", name="xdbl")
            for nt in range(HALF // NT):
                lsl = slice(nt * NT, (nt + 1) * NT)
                px = psum.tile([128, NT], F32, tag="mm", name="mm")
                for b in range(NB_I):
                    nc.tensor.matmul(px[:54, :], w["Wx"][:, b, :], xh[b][:, lsl],
                                     start=(b == 0), stop=(b == NB_I - 1))
                nc.scalar.activation(xdbl[:54, lsl], px[:54, :], AF.Copy)

            bc_dram = dram.tile([32, HALF], F16, tag="bc", name="bc_dram")
            nc.sync.dma_start(out=bc_dram, in_=xdbl[22:54, :])

            # ---- dt = softplus(xdbl[:22] @ Wdt + bdt) via Exp+Ln ----
            dt = [pool.tile([128, HALF], F16, tag=f"dt{b}", name=f"dt{b}") for b in range(NB_I)]
            for nt in range(HALF // NT):
                lsl = slice(nt * NT, (nt + 1) * NT)
                for m in range(NB_I):
                    pd = psum.tile([128, NT], F32, tag="mm", name="mm")
                    nc.tensor.matmul(pd, w["Wdt"][:, m * 128:(m + 1) * 128],
                                     xdbl[:22, lsl], start=True, stop=True)
                    et = trans.tile([128, NT], F16, tag="et", name="et")
                    nc.scalar.activation(et, pd, AF.Exp, bias=w["bdt"][:, m:m + 1])
                    nc.scalar.activation(dt[m][:, lsl], et, AF.Ln, bias=1.0)

            # ---- u = dt * xh ----
            u = [pool.tile([128, HALF], F16, tag=f"u{b}", name=f"u{b}") for b in range(NB_I)]
            for b in range(NB_I):
                nc.vector.tensor_mul(u[b], dt[b], xh[b])

            # ---- selective scan over s ----
            y = [pool.tile([128, HALF], F16, tag=f"y{b}", name=f"y{b}") for b in range(NB_I)]
            for s in range(S):
                brep = p2.tile([128, HALF], F16, tag="brep", name="brep")
                row = bc_dram[s:s + 1, :]
                nc.sync.dma_start(out=brep, in_=bass.AP(
                    tensor=row.tensor, offset=row.offset,
                    ap=[[0, 128]] + [list(d) for d in row.ap[1:]]))
                crep = p2.tile([128, HALF], F16, tag="crep", name="crep")
                row = bc_dram[16 + s:17 + s, :]
                nc.sync.dma_start(out=crep, in_=bass.AP(
                    tensor=row.tensor, offset=row.offset,
                    ap=[[0, 128]] + [list(d) for d in row.ap[1:]]))
                for b in range(NB_I):
                    dA = p2.tile([128, HALF], F16, tag="dA", name="dA")
                    nc.scalar.activation(dA, dt[b], AF.Exp, scale=float(-(s + 1)))
                    wsc = p2.tile([128, HALF], F16, tag="wsc", name="wsc")
                    nc.vector.tensor_mul(wsc, u[b], brep)
                    h = p2.tile([128, HALF], F16, tag="h", name="h")
                    init = 0.0 if half == 0 else \
                        hcarry[:, b * S + s: b * S + s + 1]
                    nc.vector.tensor_tensor_scan(h, dA, wsc, init,
                                                 OP.mult, OP.add)
                    if half == 0:
                        nc.vector.tensor_copy(
                            hcarry[:, b * S + s: b * S + s + 1],
                            h[:, HALF - 1: HALF])
                    if s == 0:
                        nc.gpsimd.tensor_mul(y[b], h, crep)
                    else:
                        g = p2.tile([128, HALF], F16, tag="g", name="g")
                        nc.gpsimd.tensor_mul(g, h, crep)
                        nc.vector.tensor_add(y[b], y[b], g)

            # ---- y = (y + xh*D) * sz ----
            for b in range(NB_I):
                y2 = trans.tile([128, HALF], F16, tag="y2", name="y2")
                nc.vector.scalar_tensor_tensor(y2, xh[b], w["Dvec"][:, b:b + 1],
                                               y[b], OP.mult, OP.add)
                nc.vector.tensor_mul(y[b], y2, sz[b])

            # ---- out_proj + residual add ----
            for nt in range(HALF // NT):
                sl = slice(h0 + nt * NT, h0 + (nt + 1) * NT)
                lsl = slice(nt * NT, (nt + 1) * NT)
                for m in range(NB_F):
                    po = psum.tile([128, NT], F32, tag="mm", name="mm")
                    for b in range(NB_I):
                        nc.tensor.matmul(po, w["Wout"][:, b, m * 128:(m + 1) * 128],
                                         y[b][:, lsl],
                                         start=(b == 0), stop=(b == NB_I - 1))
                    nc.vector.scalar_tensor_tensor(
                        resid[:, m, sl], po, w["bout"][:, m:m + 1],
                        resid[:, m, sl], OP.add, OP.add)

            # ---- LN2 ----
            h2 = pool.tile([128, NB_F, HALF], F16, tag="xn", name="xn")
            layernorm(w, "g2", "b2", half, h2)

            # ---- MLP: gelu(h2 @ Wm1 + bm1) @ (0.5*Wm2) + bm2, + resid ----
            m1 = [pool.tile([128, HALF], F16, tag=f"y{b}", name=f"y{b}") for b in range(NB_I)]
            for nt in range(HALF // NT):
                lsl = slice(nt * NT, (nt + 1) * NT)
                for m in range(NB_I):
                    pm = psum.tile([128, NT], F32, tag="mm", name="mm")
                    for b in range(NB_F):
                        k = FB[b]
                        nc.tensor.matmul(pm, w["Wm1"][:k, b, m * 128:(m + 1) * 128],
                                         h2[:k, b, lsl],
                                         start=(b == 0), stop=(b == NB_F - 1))
                    erf = trans.tile([128, NT], F16, tag="erf", name="erf")
                    nc.scalar.activation(erf, pm, AF.Erf,
                                         bias=w["bm1s"][:, m:m + 1],
                                         scale=0.7071067811865476)
                    pre = trans.tile([128, NT], F16, tag="pre", name="pre")
                    nc.scalar.activation(pre, pm, AF.Identity,
                                         bias=w["bm1"][:, m:m + 1])
                    nc.vector.scalar_tensor_tensor(m1[m][:, lsl], erf, 1.0, pre,
                                                   OP.add, OP.mult)
            for nt in range(HALF // NT):
                sl = slice(h0 + nt * NT, h0 + (nt + 1) * NT)
                lsl = slice(nt * NT, (nt + 1) * NT)
                for m in range(NB_F):
                    pm = psum.tile([128, NT], F32, tag="mm", name="mm")
                    for b in range(NB_I):
                        nc.tensor.matmul(pm, w["Wm2"][:, b, m * 128:(m + 1) * 128],
                                         m1[b][:, lsl],
                                         start=(b == 0), stop=(b == NB_I - 1))
                    nc.vector.scalar_tensor_tensor(
                        resid[:, m, sl], pm, w["bm2"][:, m:m + 1],
                        resid[:, m, sl], OP.add, OP.add)

    # ---- final sum over T ----
    osum = pool.tile([128, NB_F], F32, tag="osum", name="osum")
    for b in range(NB_F):
        nc.vector.tensor_reduce(osum[:, b:b + 1], resid[:, b, :],
                                mybir.AxisListType.X, OP.add)
    nc.sync.dma_start(out=dr["osum"][:, :], in_=osum)


def _build_program():
    nc = bacc.Bacc("TRN2", target_bir_lowering=False, debug=False, num_devices=8)
    dr = {}
    dr["inp"] = nc.dram_tensor("inp", [128, NB_F, T], F16, kind="ExternalInput")
    dr["ones128"] = nc.dram_tensor("ones128", [128, 128], F16,
                                   kind="ExternalInput")
    for name, shape, dt in [
        ("Win", [DEPTH, 128, NB_F, 1536], F16), ("bin", [DEPTH, 128, 12], F32),
        ("Wconv", [DEPTH, 128, NB_I, 4], F32), ("bconv", [DEPTH, 128, NB_I], F32),
        ("Dvec", [DEPTH, 128, NB_I], F32),
        ("Wx", [DEPTH, 128, NB_I, 54], F16),
        ("Wdt", [DEPTH, 22, 768], F16), ("bdt", [DEPTH, 128, NB_I], F32),
        ("Wout", [DEPTH, 128, NB_I, 384], F16), ("bout", [DEPTH, 128, NB_F], F32),
        ("Wm1", [DEPTH, 128, NB_F, 768], F16), ("bm1", [DEPTH, 128, NB_I], F32),
        ("bm1s", [DEPTH, 128, NB_I], F32),
        ("Wm2", [DEPTH, 128, NB_I, 384], F16), ("bm2", [DEPTH, 128, NB_F], F32),
        ("g1", [DEPTH, 128, NB_F], F32), ("b1", [DEPTH, 128, NB_F], F32),
        ("g2", [DEPTH, 128, NB_F], F32), ("b2", [DEPTH, 128, NB_F], F32),
    ]:
        dr[name] = nc.dram_tensor(name, shape, dt, kind="ExternalInput")
    dr["osum"] = nc.dram_tensor("osum", [128, NB_F], F32, kind="ExternalOutput")

    with ExitStack() as ctx:
        tc = ctx.enter_context(tile.TileContext(nc))
        _emit(ctx, tc, dr)
    nc.compile()
    return nc


_PROGRAM = None


def _get_program():
    global _PROGRAM
    if _PROGRAM is None:
        _PROGRAM = _build_program()
    return _PROGRAM


# ---------------- host-side packing ----------------

def _blockify(v, nblocks):
    out = np.zeros((128, nblocks), np.float32)
    for b in range(nblocks):
        seg = v[b * 128:(b + 1) * 128]
        out[: len(seg), b] = seg
    return out


def _kblocks(M, nblocks, ncols):
    out = np.zeros((128, nblocks, ncols), np.float32)
    for b in range(nblocks):
        seg = M[b * 128:(b + 1) * 128]
        out[: seg.shape[0], b, : seg.shape[1]] = seg
    return out


def _pack_stack(layers):
    d = {k: [] for k in ["Win", "bin", "Wconv", "bconv", "Dvec", "Wx", "Wdt",
                         "bdt", "Wout", "bout", "Wm1", "bm1", "bm1s", "Wm2",
                         "bm2", "g1", "b1", "g2", "b2"]}
    for p in layers:
        m = p["mamba"]
        in_W = np.asarray(m["in_W"], np.float32)
        in_b = np.asarray(m["in_b"], np.float32)
        Wp = np.zeros((DF, 1536), np.float32)
        Wp[:, :DI] = in_W[:, :DI]
        Wp[:, 768:768 + DI] = in_W[:, DI:]
        bp = np.zeros(1536, np.float32)
        bp[:DI] = in_b[:DI]
        bp[768:768 + DI] = in_b[DI:]
        d["Win"].append(_kblocks(Wp, NB_F, 1536))
        d["bin"].append(np.stack([bp[i * 128:(i + 1) * 128] for i in range(12)],
                                 1))
        conv_W = np.asarray(m["conv_W"], np.float32)[:, 0, :]
        d["Wconv"].append(_kblocks(conv_W, NB_I, 4))
        d["bconv"].append(_blockify(np.asarray(m["conv_b"], np.float32), NB_I))
        d["Dvec"].append(_blockify(np.asarray(m["D"], np.float32), NB_I))
        d["Wx"].append(_kblocks(np.asarray(m["x_W"], np.float32), NB_I, 54))
        wdt = np.zeros((22, 768), np.float32)
        wdt[:, :DI] = np.asarray(m["dt_W"], np.float32)
        d["Wdt"].append(wdt)
        d["bdt"].append(_blockify(np.asarray(m["dt_b"], np.float32), NB_I))
        d["Wout"].append(_kblocks(np.asarray(m["out_W"], np.float32), NB_I, 384))
        d["bout"].append(_blockify(np.asarray(m["out_b"], np.float32), NB_F))
        d["Wm1"].append(_kblocks(np.asarray(p["mlp_W1"], np.float32), NB_F, 768))
        bm1 = _blockify(np.asarray(p["mlp_b1"], np.float32), NB_I)
        d["bm1"].append(bm1)
        d["bm1s"].append(bm1 * 0.7071067811865476)
        d["Wm2"].append(_kblocks(0.5 * np.asarray(p["mlp_W2"], np.float32),
                                 NB_I, 384))
        d["bm2"].append(_blockify(np.asarray(p["mlp_b2"], np.float32), NB_F))
        d["g1"].append(_blockify(np.asarray(p["ln1_g"], np.float32), NB_F))
        d["b1"].append(_blockify(np.asarray(p["ln1_b"], np.float32), NB_F))
        d["g2"].append(_blockify(np.asarray(p["ln2_g"], np.float32), NB_F))
        d["b2"].append(_blockify(np.asarray(p["ln2_b"], np.float32), NB_F))
    f16set = {"Win", "Wx", "Wdt", "Wout", "Wm1", "Wm2"}
    return {k: np.stack(v).astype(np.float16 if k in f16set else np.float32)
            for k, v in d.items()}


def _embed(params, x, ctx):
    tok = np.asarray(params["poi_emb"], np.float32)[x]
    tim = np.asarray(params["time_emb"], np.float32)[ctx[0]]
    sp = np.stack([ctx[1], ctx[2]], -1).astype(np.float32)
    space = sp @ np.asarray(params["space_W"], np.float32) \
        + np.asarray(params["space_b"], np.float32)
    return np.concatenate([tok, tim, space], -1)


def kernel(x, context, params):
    x = np.asarray(x).astype(np.int64)
    context = np.asarray(context).astype(np.int64)
    B = x.shape[0]

    nc = _get_program()

    packed = {"f": _pack_stack(params["forw"]), "b": _pack_stack(params["back"])}
    ones128 = np.ones((128, 128), np.float16)

    in_maps = []
    for core in range(8):
        b = core % B
        fwd = core < 4
        if fwd:
            emb = _embed(params, x[b], context[:, b])
        else:
            emb = _embed(params, x[b, ::-1], context[:, b, ::-1])
        inp = np.zeros((128, NB_F, T), np.float16)
        embT = emb.T  # [352, T]
        for blk in range(NB_F):
            seg = embT[blk * 128: blk * 128 + 128]
            inp[: seg.shape[0], blk] = seg
        m = dict(packed["f" if fwd else "b"])
        m["inp"] = inp
        m["ones128"] = ones128
        in_maps.append(m)

    import os
    trace = os.environ.get("BIMAMBA_TRACE") == "1"
    try:
        res = run_bass_kernel_spmd(nc, in_maps, core_ids=list(range(8)),
                                   trace=trace)
    except ModuleNotFoundError:
        res = run_bass_kernel_spmd(nc, in_maps, core_ids=list(range(8)))
    globals()["LAST_RUN"] = res
    globals()["LAST_IN_MAPS"] = in_maps

    out = np.zeros((B, DF), np.float32)
    for bi in range(B):
        sf = res.results[bi]["osum"]
        sb = res.results[bi + 4]["osum"]
        for c in range(DF):
            out[bi, c] = (sf[c % 128, c // 128] + sb[c % 128, c // 128]) / T
    return out
